# revision 3
# baseline (speedup 1.0000x reference)
"""Trainium2 Bass kernel for nn_AppearanceLoss (keypoint patch CNN MSE).

Host: crops 33x33 patches at keypoint locations (data-dependent indices),
packs them into a quad layout, shards 256 keypoints across 8 NeuronCores.
Device: small CNN (conv 3->32 s1, conv 32->64 s2, GAP, linear 64->128)
via offset-accumulated matmuls in PSUM, partial squared-diff sums.
Host: sums 8 per-core partials into the scalar MSE.
"""

import sys

sys.path.insert(0, "/opt/trn_rl_repo")

from contextlib import ExitStack

import ml_dtypes
import numpy as np

import concourse.bass as bass  # noqa: F401
import concourse.tile as tile
from concourse import bacc, bass_utils, mybir

SIGMA = 16
PATCH = 33  # 2*SIGMA+1
HOUT = 31  # conv1 valid output: 33-3+1
COUT = 15  # conv2 stride-2 valid output: (31-3)//2+1
B, K, H = 4, 64, 256
NCORES = 8
NKP = B * K  # 256 keypoints total
KPC = NKP // NCORES  # 32 keypoints per core
NPATCH = KPC * B  # 128 patches per core per set
NQ = NPATCH // 4  # 32 quads per set
QG = 8  # quads per DMA group
BF16 = mybir.dt.bfloat16
F32 = mybir.dt.float32
NPBF16 = ml_dtypes.bfloat16

_CACHE: dict = {}


def _build_graph():
    nc = bacc.Bacc(
        "TRN2",
        target_bir_lowering=False,
        debug=False,
        enable_asserts=False,
        num_devices=NCORES,
    )
    xq_d = nc.dram_tensor(
        "xq", [12, 2 * NQ, PATCH, PATCH], BF16, kind="ExternalInput"
    ).ap()
    w1_d = nc.dram_tensor("w1", [12, 9, 128], BF16, kind="ExternalInput").ap()
    w2_d = nc.dram_tensor("w2", [64, 9, 128], BF16, kind="ExternalInput").ap()
    b1_d = nc.dram_tensor("b1", [128, 1], F32, kind="ExternalInput").ap()
    b2_d = nc.dram_tensor("b2", [64, 1], F32, kind="ExternalInput").ap()
    wl_d = nc.dram_tensor("wl", [64, 128], BF16, kind="ExternalInput").ap()
    out_d = nc.dram_tensor("out", [128, 2], F32, kind="ExternalOutput").ap()

    RELU = mybir.ActivationFunctionType.Relu
    SQUARE = mybir.ActivationFunctionType.Square

    with ExitStack() as ctx:
        tc = ctx.enter_context(tile.TileContext(nc))
        const = ctx.enter_context(tc.tile_pool(name="const", bufs=1))
        xpool = ctx.enter_context(tc.tile_pool(name="x", bufs=3))
        hpool = ctx.enter_context(tc.tile_pool(name="h", bufs=1))
        gpool = ctx.enter_context(tc.tile_pool(name="g", bufs=1))
        spool = ctx.enter_context(tc.tile_pool(name="scr", bufs=4))
        pp1 = ctx.enter_context(tc.tile_pool(name="pp1", bufs=3, space="PSUM"))
        pp2 = ctx.enter_context(tc.tile_pool(name="pp2", bufs=2, space="PSUM"))
        pp3 = ctx.enter_context(tc.tile_pool(name="pp3", bufs=1, space="PSUM"))

        w1_t = const.tile([12, 9, 128], BF16)
        nc.sync.dma_start(w1_t[:], w1_d)
        w2_t = const.tile([64, 9, 128], BF16)
        nc.sync.dma_start(w2_t[:], w2_d)
        b1_t = const.tile([128, 1], F32)
        nc.sync.dma_start(b1_t[:], b1_d)
        b2_t = const.tile([64, 1], F32)
        nc.sync.dma_start(b2_t[:], b2_d)
        wl_t = const.tile([64, 128], BF16)
        nc.sync.dma_start(wl_t[:], wl_d)

        # gap0: even pair-position patches (4q+0, 4q+2); gap1: odd (4q+1, 4q+3)
        gap0 = gpool.tile([64, 2, 2 * NQ], F32, tag="gap0")
        gap1 = gpool.tile([64, 2, 2 * NQ], F32, tag="gap1")
        res = gpool.tile([128, 2], F32, tag="res")

        for s in range(2):  # 0=ground, 1=satellite
            # conv1 outputs, split so conv2/linear rhs always starts at partition 0
            h1a = hpool.tile([64, NQ, HOUT, HOUT], BF16, tag="h1a")
            h1b = hpool.tile([64, NQ, HOUT, HOUT], BF16, tag="h1b")
            for g in range(NQ // QG):
                xg = xpool.tile([12, QG, PATCH, PATCH], BF16, tag="xg")
                nc.sync.dma_start(
                    xg[:], xq_d[:, s * NQ + g * QG : s * NQ + (g + 1) * QG]
                )
                for r in range(QG):
                    q = g * QG + r
                    for r0, nr in ((0, 16), (16, 15)):
                        ps1 = pp1.tile([128, 16, HOUT], F32, tag="ps1")
                        for o in range(9):
                            dy, dx = o // 3, o % 3
                            nc.tensor.matmul(
                                ps1[:, :nr, :],
                                w1_t[:, o, :],
                                xg[:, r, r0 + dy : r0 + dy + nr, dx : dx + HOUT],
                                start=(o == 0),
                                stop=(o == 8),
                            )
                        nc.scalar.activation(
                            h1a[:, q, r0 : r0 + nr, :],
                            ps1[0:64, :nr, :],
                            RELU,
                            bias=b1_t[0:64, :],
                        )
                        nc.scalar.activation(
                            h1b[:, q, r0 : r0 + nr, :],
                            ps1[64:128, :nr, :],
                            RELU,
                            bias=b1_t[64:128, :],
                        )
            # conv2 + GAP
            for q in range(NQ):
                ps2 = pp2.tile([128, 2, COUT * COUT], F32, tag="ps2")
                for j2, h1 in enumerate((h1a, h1b)):
                    for o in range(9):
                        dy, dx = o // 3, o % 3
                        nc.tensor.matmul(
                            ps2[:, j2, :],
                            w2_t[:, o, :],
                            h1[:, q, dy : dy + 29 : 2, dx : dx + 29 : 2],
                            start=(o == 0),
                            stop=(o == 8),
                        )
                for j2 in range(2):
                    for jj, gap in enumerate((gap0, gap1)):
                        scr = spool.tile([64, COUT * COUT], F32, tag="scr")
                        nc.scalar.activation(
                            scr[:],
                            ps2[64 * jj : 64 * jj + 64, j2, :],
                            RELU,
                            bias=b2_t[:],
                            scale=1.0 / (COUT * COUT),
                            accum_out=gap[:, s, 2 * q + j2 : 2 * q + j2 + 1],
                        )

        # linear on feature diffs (linear bias cancels in fg - fs)
        for jj, gap in enumerate((gap0, gap1)):
            dg = spool.tile([64, 2 * NQ], F32, tag=f"dg{jj}")
            nc.vector.tensor_sub(dg[:], gap[:, 0, :], gap[:, 1, :])
            dgb = spool.tile([64, 2 * NQ], BF16, tag=f"dgb{jj}")
            nc.vector.tensor_copy(dgb[:], dg[:])
            ps3 = pp3.tile([128, 2 * NQ], F32, tag=f"ps3{jj}")
            nc.tensor.matmul(ps3[:], wl_t[:], dgb[:], start=True, stop=True)
            scr3 = spool.tile([128, 2 * NQ], F32, tag=f"scr3{jj}")
            nc.scalar.activation(
                scr3[:], ps3[:], SQUARE, accum_out=res[:, jj : jj + 1]
            )
        nc.sync.dma_start(out_d, res[:])

    nc.compile()
    return nc


def _prep_weights(w1, b1, w2, b2, wl):
    w1blk = np.zeros((12, 9, 128), np.float32)
    for j in range(4):
        for c in range(3):
            for o in range(9):
                dy, dx = o // 3, o % 3
                w1blk[3 * j + c, o, 32 * j : 32 * j + 32] = w1[:, c, dy, dx]
    w2blk = np.zeros((64, 9, 128), np.float32)
    for j in range(2):
        for c in range(32):
            for o in range(9):
                dy, dx = o // 3, o % 3
                w2blk[32 * j + c, o, 64 * j : 64 * j + 64] = w2[:, c, dy, dx]
    b1q = np.tile(b1, 4)[:, None].astype(np.float32)
    b2s = (b2 / (COUT * COUT))[:, None].astype(np.float32)
    wlT = np.ascontiguousarray(wl.T)
    return (
        w1blk.astype(NPBF16),
        w2blk.astype(NPBF16),
        np.ascontiguousarray(b1q),
        np.ascontiguousarray(b2s),
        wlT.astype(NPBF16),
    )


def _crop_all(images, kps):
    # images [B,3,H,W] f32; kps [NKP,2] normalized -> patches [NKP,B,3,P,P]
    hw = images.shape[-1]
    px = kps.astype(np.float32) * np.float32(hw)
    starts = np.clip(np.floor(px).astype(np.int32) - SIGMA, 0, hw - PATCH)
    out = np.empty((kps.shape[0], images.shape[0], 3, PATCH, PATCH), np.float32)
    for n in range(kps.shape[0]):
        x, y = int(starts[n, 0]), int(starts[n, 1])
        out[n] = images[:, :, y : y + PATCH, x : x + PATCH]
    return out


def _quadize(pat):
    # [128,3,33,33] -> [12, 32, 33, 33] with partition 3j+c = patch 4q+j chan c
    return np.ascontiguousarray(
        pat.reshape(NQ, 4, 3, PATCH, PATCH)
        .transpose(1, 2, 0, 3, 4)
        .reshape(12, NQ, PATCH, PATCH)
    )


def kernel(
    images_ground,
    images_satellite,
    keypoints_ground,
    keypoints_satellite,
    w1,
    b1,
    w2,
    b2,
    wl,
    bl,
    num_samples=None,
    **_unused,
):
    images_ground = np.asarray(images_ground, np.float32)
    images_satellite = np.asarray(images_satellite, np.float32)
    kg = np.asarray(keypoints_ground, np.float32).reshape(-1, 2)
    ks = np.asarray(keypoints_satellite, np.float32).reshape(-1, 2)
    w1 = np.asarray(w1, np.float32)
    b1 = np.asarray(b1, np.float32)
    w2 = np.asarray(w2, np.float32)
    b2 = np.asarray(b2, np.float32)
    wl = np.asarray(wl, np.float32)

    pg = _crop_all(images_ground, kg)  # [256,4,3,33,33]
    ps = _crop_all(images_satellite, ks)

    w1blk, w2blk, b1q, b2s, wlT = _prep_weights(w1, b1, w2, b2, wl)

    if "nc" not in _CACHE:
        _CACHE["nc"] = _build_graph()
    nc = _CACHE["nc"]

    in_maps = []
    for i in range(NCORES):
        sl = slice(i * KPC, (i + 1) * KPC)
        patg = pg[sl].reshape(NPATCH, 3, PATCH, PATCH)
        pats = ps[sl].reshape(NPATCH, 3, PATCH, PATCH)
        xq = np.concatenate([_quadize(patg), _quadize(pats)], axis=1).astype(NPBF16)
        in_maps.append(
            dict(xq=xq, w1=w1blk, w2=w2blk, b1=b1q, b2=b2s, wl=wlT)
        )

    results = bass_utils.run_bass_kernel_spmd(
        nc, in_maps, core_ids=list(range(NCORES))
    )
    total = np.float64(0.0)
    for r in results.results:
        total += np.asarray(r["out"], np.float64).sum()
    mse = total / (NKP * B * 128)
    return np.asarray(mse, np.float32)


if __name__ == "__main__":
    rng = np.random.default_rng(0)
    ins = dict(
        images_ground=rng.standard_normal((B, 3, H, H), np.float32),
        images_satellite=rng.standard_normal((B, 3, H, H), np.float32),
        keypoints_ground=(0.2 + 0.6 * rng.random((B, K, 2))).astype(np.float32),
        keypoints_satellite=(0.2 + 0.6 * rng.random((B, K, 2))).astype(np.float32),
        w1=(rng.standard_normal((32, 3, 3, 3), np.float32) * 0.1),
        b1=np.zeros(32, np.float32),
        w2=(rng.standard_normal((64, 32, 3, 3), np.float32) * 0.05),
        b2=np.zeros(64, np.float32),
        wl=(rng.standard_normal((128, 64), np.float32) * 0.1),
        bl=np.zeros(128, np.float32),
        num_samples=K,
    )
    print("kernel out:", kernel(**ins))


# revision 14
# speedup vs baseline: 1.7192x; 1.7192x over previous
"""Trainium2 Bass kernel for nn_AppearanceLoss (keypoint patch CNN MSE).

Host: crops 33x33 patches at keypoint locations (data-dependent indices),
packs them into a quad layout, shards 256 keypoints across 8 NeuronCores.
Device: small CNN (conv 3->32 s1, conv 32->64 s2, GAP, linear 64->128)
via offset-accumulated matmuls in PSUM with tile_position row-packing
(4 concurrent quads in conv1, 2 concurrent pairs in conv2), partial
squared-diff sums. Host: sums 8 per-core partials into the scalar MSE.
"""

import sys

sys.path.insert(0, "/opt/trn_rl_repo")

from contextlib import ExitStack

import ml_dtypes
import numpy as np

import concourse.bass as bass  # noqa: F401
import concourse.tile as tile
from concourse import bacc, bass_utils, mybir

SIGMA = 16
PATCH = 33  # 2*SIGMA+1
HOUT = 31  # conv1 valid output: 33-3+1
COUT = 15  # conv2 stride-2 valid output: (31-3)//2+1
B, K, H = 4, 64, 256
NCORES = 8
NKP = B * K  # 256 keypoints total
KPC = NKP // NCORES  # 32 keypoints per core
NPATCH = KPC * B  # 128 patches per core per set
NQ = NPATCH // 4  # 32 quads per set
NG = NQ // 4  # 8 groups of 4 quads per set; 16 groups total (both sets)
NQT = 2 * NQ  # 64 quads total per core
BF16 = mybir.dt.bfloat16
F32 = mybir.dt.float32
NPBF16 = ml_dtypes.bfloat16

_CACHE: dict = {}


def _build_graph():
    nc = bacc.Bacc(
        "TRN2",
        target_bir_lowering=False,
        debug=False,
        enable_asserts=False,
        num_devices=NCORES,
    )
    # compact patch layout: row 12r+3j+c, col = group g (0..15), quad = 4g+r
    xq_d = nc.dram_tensor(
        "xq", [48, 2 * NG, PATCH, PATCH], BF16, kind="ExternalInput"
    ).ap()
    w1_d = nc.dram_tensor("w1", [128, 9, 128], BF16, kind="ExternalInput").ap()
    w2_d = nc.dram_tensor("w2", [128, 9, 128], BF16, kind="ExternalInput").ap()
    b1_d = nc.dram_tensor("b1", [128, 1], F32, kind="ExternalInput").ap()
    b2_d = nc.dram_tensor("b2", [128, 1], F32, kind="ExternalInput").ap()
    wl_d = nc.dram_tensor("wl", [128, 128], BF16, kind="ExternalInput").ap()
    out_d = nc.dram_tensor("out", [128, 2], F32, kind="ExternalOutput").ap()

    RELU = mybir.ActivationFunctionType.Relu
    SQUARE = mybir.ActivationFunctionType.Square

    with ExitStack() as ctx:
        tc = ctx.enter_context(tile.TileContext(nc))
        const = ctx.enter_context(tc.tile_pool(name="const", bufs=1))
        xpool = ctx.enter_context(tc.tile_pool(name="x", bufs=1))
        hpool = ctx.enter_context(tc.tile_pool(name="h", bufs=1))
        gpool = ctx.enter_context(tc.tile_pool(name="g", bufs=1))
        spool = ctx.enter_context(tc.tile_pool(name="scr", bufs=4))
        pp1 = ctx.enter_context(tc.tile_pool(name="pp1", bufs=4, space="PSUM"))
        pp2 = ctx.enter_context(tc.tile_pool(name="pp2", bufs=4, space="PSUM"))

        w1_t = const.tile([128, 9, 128], BF16)
        nc.sync.dma_start(w1_t[:], w1_d)
        w2_t = const.tile([128, 9, 128], BF16)
        nc.sync.dma_start(w2_t[:], w2_d)
        b1_t = const.tile([128, 1], F32)
        nc.sync.dma_start(b1_t[:], b1_d)
        b2_t = const.tile([128, 1], F32)
        nc.sync.dma_start(b2_t[:], b2_d)
        wl_t = const.tile([128, 128], BF16)
        nc.sync.dma_start(wl_t[:], wl_d)

        # patches: partition 32r+3j+c = quad 4g+r, patch-in-quad j, channel c
        xsb = xpool.tile([128, 2 * NG, PATCH, PATCH], BF16)
        for r in range(4):
            nc.sync.dma_start(
                xsb[32 * r : 32 * r + 12, :], xq_d[12 * r : 12 * r + 12, :]
            )

        # conv1 out: partition 32j+m = patch-in-quad j, channel m
        h1 = hpool.tile([128, NQT, HOUT, HOUT], BF16)
        # gap col 2q+j2; partition 64a+m = patch (q, 2*j2+a) channel m
        gap = gpool.tile([128, NQT * 2], F32)
        res = gpool.tile([128, 2], F32)

        # ---- conv1: 4 concurrent row-tiles (one quad each, K=12) ----
        for g in range(2 * NG):
            for r0, nr in ((0, 16), (16, 15)):
                pss = [
                    pp1.tile([128, 16, HOUT], F32, tag="ps1", name=f"ps1_{r}")
                    for r in range(4)
                ]
                for o in range(9):
                    dy, dx = o // 3, o % 3
                    for r in range(4):
                        p0 = 32 * r
                        nc.tensor.matmul(
                            pss[r][:, :nr, :],
                            w1_t[p0 : p0 + 12, o, :],
                            xsb[p0 : p0 + 12, g, r0 + dy : r0 + dy + nr, dx : dx + HOUT],
                            start=(o == 0),
                            stop=(o == 8),
                            tile_position=(p0, 0),
                        )
                for r in range(4):
                    nc.scalar.activation(
                        h1[:, 4 * g + r, r0 : r0 + nr, :],
                        pss[r][:, :nr, :],
                        RELU,
                        bias=b1_t[:],
                    )

        # ---- conv2: 2 concurrent row-tiles (pairs, K=64), 2-quad groups ----
        # one psum tile (bank) per accumulation group: same-bank interleaved
        # groups are illegal (zero-region conflict -> device crash)
        for G in range(NQT // 2):
            ps2s = [
                [
                    pp2.tile(
                        [128, COUT * COUT], F32, tag="ps2", name=f"ps2_{k}_{jj}"
                    )
                    for jj in range(2)
                ]
                for k in range(2)
            ]
            for o in range(9):
                dy, dx = o // 3, o % 3
                for jj in range(2):
                    p0 = 64 * jj
                    for k in range(2):
                        q = 2 * G + k
                        nc.tensor.matmul(
                            ps2s[k][jj][:],
                            w2_t[p0 : p0 + 64, o, :],
                            h1[p0 : p0 + 64, q, dy : dy + 29 : 2, dx : dx + 29 : 2],
                            start=(o == 0),
                            stop=(o == 8),
                            tile_position=(p0, 0),
                        )
            for k in range(2):
                q = 2 * G + k
                for j2 in range(2):
                    scr = spool.tile([128, COUT * COUT], F32, tag="scr")
                    nc.scalar.activation(
                        scr[:],
                        ps2s[k][j2][:],
                        RELU,
                        bias=b2_t[:],
                        scale=1.0 / (COUT * COUT),
                        accum_out=gap[:, 2 * q + j2 : 2 * q + j2 + 1],
                    )

        # ---- linear on feature diffs (bias cancels), squared sums ----
        dg = spool.tile([128, NQ * 2], F32, tag="dg")
        nc.vector.tensor_sub(dg[:], gap[:, 0 : NQ * 2], gap[:, NQ * 2 : NQT * 2])
        dgb = spool.tile([128, NQ * 2], BF16, tag="dgb")
        nc.vector.tensor_copy(dgb[:], dg[:])
        for jj in range(2):
            p0 = 64 * jj
            ps3 = pp2.tile([128, NQ * 2], F32, tag="ps2", name=f"ps3_{jj}")
            nc.tensor.matmul(
                ps3[:],
                wl_t[p0 : p0 + 64, :],
                dgb[p0 : p0 + 64, :],
                start=True,
                stop=True,
                tile_position=(p0, 0),
            )
            scr3 = spool.tile([128, NQ * 2], F32, tag="scr3", name=f"scr3_{jj}")
            nc.scalar.activation(
                scr3[:], ps3[:], SQUARE, accum_out=res[:, jj : jj + 1]
            )
        nc.sync.dma_start(out_d, res[:])

    nc.compile()
    return nc


def _prep_weights(w1, b1, w2, b2, wl):
    w1blk = np.zeros((128, 9, 128), np.float32)
    for r in range(4):
        for j in range(4):
            for c in range(3):
                for o in range(9):
                    dy, dx = o // 3, o % 3
                    w1blk[32 * r + 3 * j + c, o, 32 * j : 32 * j + 32] = w1[
                        :, c, dy, dx
                    ]
    w2blk = np.zeros((128, 9, 128), np.float32)
    for jj in range(2):
        for j in range(2):
            for c in range(32):
                for o in range(9):
                    dy, dx = o // 3, o % 3
                    w2blk[64 * jj + 32 * j + c, o, 64 * j : 64 * j + 64] = w2[
                        :, c, dy, dx
                    ]
    b1q = np.tile(b1, 4)[:, None].astype(np.float32)
    b2q = np.tile(b2 / (COUT * COUT), 2)[:, None].astype(np.float32)
    wlrep = np.zeros((128, 128), np.float32)
    wlrep[0:64] = wl.T
    wlrep[64:128] = wl.T
    return (
        w1blk.astype(NPBF16),
        w2blk.astype(NPBF16),
        np.ascontiguousarray(b1q),
        np.ascontiguousarray(b2q),
        wlrep.astype(NPBF16),
    )


def _crop_all(images, kps):
    # images [B,3,H,W] f32; kps [NKP,2] normalized -> patches [NKP,B,3,P,P]
    hw = images.shape[-1]
    px = kps.astype(np.float32) * np.float32(hw)
    starts = np.clip(np.floor(px).astype(np.int32) - SIGMA, 0, hw - PATCH)
    out = np.empty((kps.shape[0], images.shape[0], 3, PATCH, PATCH), np.float32)
    for n in range(kps.shape[0]):
        x, y = int(starts[n, 0]), int(starts[n, 1])
        out[n] = images[:, :, y : y + PATCH, x : x + PATCH]
    return out


def _quadize(pat):
    # [128,3,33,33] -> [48, 8, 33, 33]; row 12r+3j+c, col g, patch 16g+4r+j
    return np.ascontiguousarray(
        pat.reshape(NG, 4, 4, 3, PATCH, PATCH)
        .transpose(1, 2, 3, 0, 4, 5)
        .reshape(48, NG, PATCH, PATCH)
    )


def kernel(
    images_ground,
    images_satellite,
    keypoints_ground,
    keypoints_satellite,
    w1,
    b1,
    w2,
    b2,
    wl,
    bl,
    num_samples=None,
    **_unused,
):
    images_ground = np.asarray(images_ground, np.float32)
    images_satellite = np.asarray(images_satellite, np.float32)
    kg = np.asarray(keypoints_ground, np.float32).reshape(-1, 2)
    ks = np.asarray(keypoints_satellite, np.float32).reshape(-1, 2)
    w1 = np.asarray(w1, np.float32)
    b1 = np.asarray(b1, np.float32)
    w2 = np.asarray(w2, np.float32)
    b2 = np.asarray(b2, np.float32)
    wl = np.asarray(wl, np.float32)

    pg = _crop_all(images_ground, kg)  # [256,4,3,33,33]
    ps = _crop_all(images_satellite, ks)

    w1blk, w2blk, b1q, b2q, wlrep = _prep_weights(w1, b1, w2, b2, wl)

    if "nc" not in _CACHE:
        _CACHE["nc"] = _build_graph()
    nc = _CACHE["nc"]

    in_maps = []
    for i in range(NCORES):
        sl = slice(i * KPC, (i + 1) * KPC)
        patg = pg[sl].reshape(NPATCH, 3, PATCH, PATCH)
        pats = ps[sl].reshape(NPATCH, 3, PATCH, PATCH)
        xq = np.concatenate([_quadize(patg), _quadize(pats)], axis=1).astype(NPBF16)
        in_maps.append(dict(xq=xq, w1=w1blk, w2=w2blk, b1=b1q, b2=b2q, wl=wlrep))

    results = bass_utils.run_bass_kernel_spmd(
        nc, in_maps, core_ids=list(range(NCORES))
    )
    total = np.float64(0.0)
    for r in results.results:
        total += np.asarray(r["out"], np.float64).sum()
    mse = total / (NKP * B * 128)
    return np.asarray(mse, np.float32)


if __name__ == "__main__":
    rng = np.random.default_rng(0)
    ins = dict(
        images_ground=rng.standard_normal((B, 3, H, H), np.float32),
        images_satellite=rng.standard_normal((B, 3, H, H), np.float32),
        keypoints_ground=(0.2 + 0.6 * rng.random((B, K, 2))).astype(np.float32),
        keypoints_satellite=(0.2 + 0.6 * rng.random((B, K, 2))).astype(np.float32),
        w1=(rng.standard_normal((32, 3, 3, 3), np.float32) * 0.1),
        b1=np.zeros(32, np.float32),
        w2=(rng.standard_normal((64, 32, 3, 3), np.float32) * 0.05),
        b2=np.zeros(64, np.float32),
        wl=(rng.standard_normal((128, 64), np.float32) * 0.1),
        bl=np.zeros(128, np.float32),
        num_samples=K,
    )
    print("kernel out:", kernel(**ins))
